# revision 20
# baseline (speedup 1.0000x reference)
"""EdgeDecoder Trainium2 kernel: out = relu(concat(z_user[row], z_item[col]) @ W1 + b1) @ W2 + b2.

Strategy (8 NeuronCores, SPMD), default variant "sel":
  - 2D shard of the EDGE list by endpoint id range: 4 user ranges x 2 item
    ranges -> 8 cores; each core sees a 25000-row slice of each table, so
    item indices fit dma_gather's int16.
  - Device pipeline per core:
      1. TensorE precomputes per-node tables (PE transpose + matmul):
         V' = zi @ W1b -> bf16 rows in DRAM; U' = zu @ W1a + b1 -> bf16,
         one SBUF tile per 128-user "window" (fine-grained deps let the
         scheduler overlap U' precompute with the first gathers).
      2. ITEM side: per 1024-edge dma_gather (HW cap) pulls each edge's V'
         row into edge-major [128, k, 128] bf16 tiles (~9 ns/index, SWDGE
         descriptor-emission bound - the kernel's dominant cost).
      3. USER side needs NO indices: the host bin-packs users into 210
         windows (<=128 users, <=tpw*128 edges each) and streams a one-hot
         matrix per 128-edge tile; TensorE "selection matmuls"
         (one-hot^T @ U'window) materialize each edge's U' row. A fixed
         window-per-tile schedule keeps the graph SPMD-uniform.
      4. DVE: t = relu(Ug + Vg); multiply by W2 (broadcast, fp16) and
         pairwise tree-reduce the hidden dim (tensor_tensor runs 2x on
         16-bit; tensor_reduce would be 1x).
      5. add b2 and DMA scores out per chunk (partition-major layout;
         host restores edge order).
  - Host does sharding, bin-packing, index/one-hot formatting and the
    inverse permutation; all FLOPs (casts, matmuls, bias, relu, reduce)
    run on device.
  - TRN_KERNEL_VARIANT=gather selects the simpler both-sides-gathered
    fallback (~2.6 ms); the default sel variant measures ~1.35 ms.
"""

import os
import numpy as np

NUM_USERS = 100000
NUM_ITEMS = 50000
HIDDEN = 128
N_CORES = 8
U_SPLIT, I_SPLIT = 4, 2
U_RANGE = NUM_USERS // U_SPLIT  # 25000
I_RANGE = NUM_ITEMS // I_SPLIT  # 25000
GCHUNK = 1024                   # edges per dma_gather call (HW cap ~1024)
CHUNK = 4096                    # edges per compute chunk
T_PAD = 25088                   # table rows padded to 128 multiple

LAST_EXEC_TIME_NS = None
LAST_RESULTS = None


def _maybe_install_ntff_hook():
    """Register the NTFF profiling hook if the boot module is present."""
    import sys, types
    if "antenv.axon_hooks" in sys.modules:
        return
    try:
        import antenv
        from trn_agent_boot.trn_boot import _ntff_profile_via_ctypes
    except Exception:
        return
    mod = types.ModuleType("antenv.axon_hooks")
    state = {"hook": None}
    mod.set_axon_ntff_profile_hook = lambda h: state.__setitem__("hook", h)
    mod.get_axon_ntff_profile_hook = lambda: state["hook"]
    sys.modules["antenv.axon_hooks"] = mod
    antenv.axon_hooks = mod
    try:
        mod.set_axon_ntff_profile_hook(
            _ntff_profile_via_ctypes("/opt/axon/libaxon_pjrt.so"))
    except Exception:
        pass


def _build(e_pad: int):
    import concourse.bacc as bacc
    import concourse.mybir as mybir
    import concourse.tile as tile
    from concourse.masks import make_identity

    nc = bacc.Bacc("TRN2", target_bir_lowering=False, debug=True)
    f32, bf16, fp16, i16 = (mybir.dt.float32, mybir.dt.bfloat16,
                            mybir.dt.float16, mybir.dt.int16)
    H = HIDDEN

    zu = nc.declare_dram_parameter("zu", [T_PAD, H], f32, isOutput=False)
    zi = nc.declare_dram_parameter("zi", [T_PAD, H], f32, isOutput=False)
    uidx = nc.declare_dram_parameter("uidx", [128, e_pad // 16], i16, isOutput=False)
    vidx = nc.declare_dram_parameter("vidx", [128, e_pad // 16], i16, isOutput=False)
    w1 = nc.declare_dram_parameter("w1", [2 * H, H], f32, isOutput=False)
    b1r = nc.declare_dram_parameter("b1r", [1, H], f32, isOutput=False)
    w2r = nc.declare_dram_parameter("w2r", [128, H], f32, isOutput=False)
    b2r = nc.declare_dram_parameter("b2r", [128, 1], f32, isOutput=False)
    out = nc.declare_dram_parameter("out", [128, e_pad // 128], f32, isOutput=True)

    ut = nc.dram_tensor("ut", [T_PAD, H], bf16)
    vt = nc.dram_tensor("vt", [T_PAD, H], bf16)

    n_chunks = e_pad // CHUNK
    kc = CHUNK // 128          # rows per chunk in edge-major layout
    n_tiles = T_PAD // 128

    with tile.TileContext(nc) as tc:
        with (
            tc.tile_pool(name="consts", bufs=1) as consts,
            tc.tile_pool(name="pc", bufs=3) as pc_pool,
            tc.tile_pool(name="idx", bufs=1) as idx_pool,
            tc.tile_pool(name="gather", bufs=4) as g_pool,
            tc.tile_pool(name="ep", bufs=2) as ep_pool,
            tc.tile_pool(name="res", bufs=1) as res_pool,
            tc.tile_pool(name="ps_t", bufs=2, space="PSUM") as pst_pool,
            tc.tile_pool(name="ps_o", bufs=2, space="PSUM") as pso_pool,
        ):
            # ---- constants ----
            w1a = consts.tile([128, H], bf16, tag="w1a")
            w1b = consts.tile([128, H], bf16, tag="w1b")
            nc.gpsimd.dma_start(out=w1a[:], in_=w1[0:H, :])
            nc.gpsimd.dma_start(out=w1b[:], in_=w1[H:2 * H, :])
            b1row = consts.tile([1, H], f32, tag="b1row")
            nc.sync.dma_start(out=b1row[:], in_=b1r[:])
            ones1 = consts.tile([1, 128], bf16, tag="ones1")
            nc.vector.memset(ones1[:], 1.0)
            b1b = consts.tile([1, H], bf16, tag="b1b")
            nc.vector.tensor_copy(out=b1b[:], in_=b1row[:])
            w2b = consts.tile([128, H], bf16, tag="w2b")
            nc.gpsimd.dma_start(out=w2b[:], in_=w2r[:])
            b2t = consts.tile([128, 1], f32, tag="b2t")
            nc.sync.dma_start(out=b2t[:], in_=b2r[:])
            ident = consts.tile([128, 128], f32, tag="ident")
            make_identity(nc, ident[:])

            # ---- phase 1: precompute tables U' = zu@W1a + b1, V' = zi@W1b ----
            B = 4  # batched node tiles per DMA
            for src, dst, wx, with_bias in ((zu, ut, w1a, True),
                                            (zi, vt, w1b, False)):
                src_b = src[:].rearrange("(n t p) d -> n p t d", t=B, p=128)
                dst_b = dst[:].rearrange("(n t p) d -> n p t d", t=B, p=128)
                for bi in range(n_tiles // B):
                    zt = pc_pool.tile([128, B, H], f32, tag="zt")
                    nc.sync.dma_start(out=zt[:], in_=src_b[bi])
                    ub = pc_pool.tile([128, B, H], bf16, tag="ub")
                    for t in range(B):
                        ztp = pst_pool.tile([128, 128], f32, tag="ztp")
                        nc.tensor.transpose(out=ztp[:], in_=zt[:, t, :],
                                            identity=ident[:])
                        ztb = pc_pool.tile([128, 128], bf16, tag="ztb")
                        nc.scalar.copy(out=ztb[:], in_=ztp[:])
                        up = pso_pool.tile([128, H], f32, tag="up")
                        nc.tensor.matmul(up[:], ztb[:], wx[:],
                                         start=True, stop=not with_bias)
                        if with_bias:
                            nc.tensor.matmul(up[:], ones1[:], b1b[:],
                                             start=False, stop=True)
                        nc.scalar.copy(out=ub[:, t, :], in_=up[:])
                    nc.sync.dma_start(out=dst_b[bi], in_=ub[:])

            # ---- index arrays resident in SBUF ----
            uix = idx_pool.tile([128, e_pad // 16], i16, tag="uix")
            vix = idx_pool.tile([128, e_pad // 16], i16, tag="vix")
            nc.sync.dma_start(out=uix[:], in_=uidx[:])
            nc.sync.dma_start(out=vix[:], in_=vidx[:])

            # result accumulator [128, e_pad/128] f32 (edge e -> [e%128, e//128])
            racc = res_pool.tile([128, e_pad // 128], f32, tag="racc")

            # ---- phase 2: gather + epilogue per chunk ----
            for c in range(n_chunks):
                ug = g_pool.tile([128, kc, H], bf16, tag="g")
                vg = g_pool.tile([128, kc, H], bf16, tag="g")
                for gi in range(CHUNK // GCHUNK):
                    i0 = (c * CHUNK + gi * GCHUNK) // 16
                    o0 = gi * (GCHUNK // 128)
                    nc.gpsimd.dma_gather(
                        ug[:, o0:o0 + GCHUNK // 128, :], ut[:],
                        uix[:, i0:i0 + GCHUNK // 16],
                        num_idxs=GCHUNK, num_idxs_reg=GCHUNK,
                        elem_size=H, elem_step=H, transpose=False)
                    nc.gpsimd.dma_gather(
                        vg[:, o0:o0 + GCHUNK // 128, :], vt[:],
                        vix[:, i0:i0 + GCHUNK // 16],
                        num_idxs=GCHUNK, num_idxs_reg=GCHUNK,
                        elem_size=H, elem_step=H, transpose=False)

                # t = relu(ug + vg)  (bf16 2x add, then 4x max-with-0)
                t = ep_pool.tile([128, kc, H], bf16, tag="t")
                nc.vector.tensor_tensor(out=t[:], in0=ug[:], in1=vg[:],
                                        op=mybir.AluOpType.add)
                nc.vector.tensor_scalar_max(out=t[:], in0=t[:], scalar1=0.0)
                # m = t * w2 (fp16 out for accurate tree reduce)
                m = ep_pool.tile([128, kc, H], fp16, tag="m")
                from concourse.bass import AP as _AP
                w2bc = _AP(w2b[:].tensor, w2b[:].offset, [[H, 128], [0, kc], [1, H]])
                nc.vector.tensor_tensor(
                    out=m[:], in0=t[:], in1=w2bc,
                    op=mybir.AluOpType.mult)
                # pairwise tree reduce over hidden (innermost) dim
                w = H
                while w > 2:
                    half = w // 2
                    nc.vector.tensor_tensor(
                        out=m[:, :, 0:half], in0=m[:, :, 0:half],
                        in1=m[:, :, half:w], op=mybir.AluOpType.add)
                    w = half
                nc.vector.tensor_tensor(
                    out=racc[:, c * kc:(c + 1) * kc],
                    in0=m[:, :, 0], in1=m[:, :, 1], op=mybir.AluOpType.add)


            # add b2, write out
            nc.vector.tensor_scalar_add(out=racc[:], in0=racc[:], scalar1=b2t[:, 0:1])
            nc.sync.dma_start(out=out[:], in_=racc[:])

    nc.compile()
    return nc



# ---- v2b: U-side via PE selection-matmul (zero gather indices), V-side gathered ----
W_WIN = 210          # user windows per core (bin-packed), table rows = W_WIN*128
T_PAD2 = W_WIN * 128  # 26880
TPW = 5              # tiles (of 128 edges) per window in the fixed schedule
N_EXTRA_CH = 0       # computed at build


def _build_sel(e_pad: int, wid_of_tile):
    import concourse.bacc as bacc
    import concourse.mybir as mybir
    import concourse.tile as tile
    from concourse.masks import make_identity

    nc = bacc.Bacc("TRN2", target_bir_lowering=False, debug=True)
    f32, bf16, fp16, i16 = (mybir.dt.float32, mybir.dt.bfloat16,
                            mybir.dt.float16, mybir.dt.int16)
    H = HIDDEN
    n_tiles_e = e_pad // 128

    zu = nc.declare_dram_parameter("zu", [T_PAD2, H], f32, isOutput=False)
    zi = nc.declare_dram_parameter("zi", [T_PAD, H], f32, isOutput=False)
    vidx = nc.declare_dram_parameter("vidx", [128, e_pad // 16], i16, isOutput=False)
    oh = nc.declare_dram_parameter("oh", [n_tiles_e, 128, 128], bf16, isOutput=False)
    w1 = nc.declare_dram_parameter("w1", [2 * H, H], f32, isOutput=False)
    b1r = nc.declare_dram_parameter("b1r", [1, H], f32, isOutput=False)
    w2r = nc.declare_dram_parameter("w2r", [128, H], f32, isOutput=False)
    b2r = nc.declare_dram_parameter("b2r", [128, 1], f32, isOutput=False)
    out = nc.declare_dram_parameter("out", [128, e_pad // 128], f32, isOutput=True)

    vt = nc.dram_tensor("vt", [T_PAD, H], bf16)

    n_chunks = e_pad // CHUNK
    kc = CHUNK // 128

    with tile.TileContext(nc) as tc:
        with (
            tc.tile_pool(name="consts", bufs=1) as consts,
            tc.tile_pool(name="pc", bufs=3) as pc_pool,
            tc.tile_pool(name="idx", bufs=1) as idx_pool,
            tc.tile_pool(name="gather", bufs=4) as g_pool,
            tc.tile_pool(name="ohp", bufs=2) as oh_pool,
            tc.tile_pool(name="ub", bufs=2) as ub_pool,
            tc.tile_pool(name="ep", bufs=2) as ep_pool,
            tc.tile_pool(name="res", bufs=1) as res_pool,
            tc.tile_pool(name="ps_t", bufs=2, space="PSUM") as pst_pool,
            tc.tile_pool(name="ps_o", bufs=2, space="PSUM") as pso_pool,
            tc.tile_pool(name="ps_s", bufs=2, space="PSUM") as pss_pool,
        ):
            # ---- constants ----
            w1a = consts.tile([128, H], bf16, tag="w1a")
            w1b = consts.tile([128, H], bf16, tag="w1b")
            nc.gpsimd.dma_start(out=w1a[:], in_=w1[0:H, :])
            nc.gpsimd.dma_start(out=w1b[:], in_=w1[H:2 * H, :])
            b1row = consts.tile([1, H], f32, tag="b1row")
            nc.sync.dma_start(out=b1row[:], in_=b1r[:])
            ones1 = consts.tile([1, 128], bf16, tag="ones1")
            nc.vector.memset(ones1[:], 1.0)
            b1b = consts.tile([1, H], bf16, tag="b1b")
            nc.vector.tensor_copy(out=b1b[:], in_=b1row[:])
            w2b = consts.tile([128, H], bf16, tag="w2b")
            nc.gpsimd.dma_start(out=w2b[:], in_=w2r[:])
            b2t = consts.tile([128, 1], f32, tag="b2t")
            nc.sync.dma_start(out=b2t[:], in_=b2r[:])
            ident = consts.tile([128, 128], f32, tag="ident")
            make_identity(nc, ident[:])

            # U' table: one SBUF tile per window (fine-grained deps so the
            # scheduler overlaps U' precompute with the first gathers)
            usb_w = [consts.tile([128, H], bf16, name=f"usb{w}", tag=f"usb{w}")
                     for w in range(W_WIN)]

            # ---- precompute V' -> DRAM (first: gathers depend on it) ----
            # cast zi -> bf16 (SWDGE), then HWDGE transpose-loads feed the
            # W1b matmuls directly (no PE transposes, no PSUM->SBUF hop)
            B = 4
            n_tiles_v = T_PAD // 128
            zib = nc.dram_tensor("zib", [T_PAD, H], bf16)
            per_p = T_PAD * H // 128
            zi_f = zi[:].rearrange("r d -> (r d)").rearrange("(p a) -> p a", p=128)
            zib_f = zib[:].rearrange("r d -> (r d)").rearrange("(p a) -> p a", p=128)
            n_sl = 8
            sl = per_p // n_sl
            for s in range(n_sl):
                tcst = pc_pool.tile([128, sl], bf16, tag="tcst", bufs=2)
                nc.gpsimd.dma_start(out=tcst[:], in_=zi_f[:, s * sl:(s + 1) * sl])
                nc.scalar.dma_start(out=zib_f[:, s * sl:(s + 1) * sl], in_=tcst[:])
            vt_b = vt[:].rearrange("(n t p) d -> n p t d", t=B, p=128)
            for bi in range(n_tiles_v // B):
                ztb = pc_pool.tile([128, B * 128], bf16, tag="ztb")
                nc.sync.dma_start(out=ztb[:], in_=zib[bi * B * 128:(bi + 1) * B * 128, :],
                                  transpose=True)
                up = pso_pool.tile([128, B, H], f32, tag="up")
                for t in range(B):
                    nc.tensor.matmul(up[:, t, :], ztb[:, t * 128:(t + 1) * 128],
                                     w1b[:], start=True, stop=True)
                ub = pc_pool.tile([128, B, H], bf16, tag="ubv")
                if bi % 2 == 0:
                    nc.vector.tensor_copy(out=ub[:], in_=up[:])
                else:
                    nc.scalar.copy(out=ub[:], in_=up[:])
                nc.sync.dma_start(out=vt_b[bi], in_=ub[:])

            # ---- precompute U' into per-window SBUF tiles ----
            nb_u = (W_WIN // B) * B * 128
            zu_b = zu[0:nb_u, :].rearrange("(n t p) d -> n p t d", t=B, p=128)
            def u_window(w, zt_ap):
                ztp = pst_pool.tile([128, 128], f32, tag="ztp")
                nc.tensor.transpose(out=ztp[:], in_=zt_ap, identity=ident[:])
                ztb = pc_pool.tile([128, 128], bf16, tag="ztb")
                if w % 2 == 0:
                    nc.scalar.copy(out=ztb[:], in_=ztp[:])
                else:
                    nc.vector.tensor_copy(out=ztb[:], in_=ztp[:])
                up = pso_pool.tile([128, H], f32, tag="up")
                nc.tensor.matmul(up[:], ztb[:], w1a[:], start=True, stop=False)
                nc.tensor.matmul(up[:], ones1[:], b1b[:], start=False, stop=True)
                if w % 2 == 0:
                    nc.vector.tensor_copy(out=usb_w[w][:], in_=up[:])
                else:
                    nc.scalar.copy(out=usb_w[w][:], in_=up[:])
            for bi in range(W_WIN // B):
                zt = pc_pool.tile([128, B, H], f32, tag="zt")
                nc.sync.dma_start(out=zt[:], in_=zu_b[bi])
                for t in range(B):
                    u_window(bi * B + t, zt[:, t, :])
            for w in range((W_WIN // B) * B, W_WIN):
                zt1 = pc_pool.tile([128, H], f32, tag="zt1")
                nc.sync.dma_start(out=zt1[:], in_=zu[w * 128:(w + 1) * 128, :])
                u_window(w, zt1[:])

            # ---- index array resident in SBUF ----
            vix = idx_pool.tile([128, e_pad // 16], i16, tag="vix")
            nc.sync.dma_start(out=vix[:], in_=vidx[:])

            racc = res_pool.tile([128, e_pad // 128], f32, tag="racc")
            oh_r = oh[:].rearrange("t l e -> l t e")

            # ---- main loop ----
            for c in range(n_chunks):
                t0 = c * kc
                ohs = oh_pool.tile([128, kc, 128], bf16, tag="ohs")
                nc.sync.dma_start(out=ohs[:], in_=oh_r[:, t0:t0 + kc, :])
                vg = g_pool.tile([128, kc, H], bf16, tag="g")
                for gi in range(CHUNK // GCHUNK):
                    i0 = (c * CHUNK + gi * GCHUNK) // 16
                    o0 = gi * (GCHUNK // 128)
                    nc.gpsimd.dma_gather(
                        vg[:, o0:o0 + GCHUNK // 128, :], vt[:],
                        vix[:, i0:i0 + GCHUNK // 16],
                        num_idxs=GCHUNK, num_idxs_reg=GCHUNK,
                        elem_size=H, elem_step=H, transpose=False)

                ub = ub_pool.tile([128, kc, H], bf16, tag="ubm")
                for q in range(kc // 4):
                    ps = pss_pool.tile([128, 4, H], f32, tag="pss")
                    for j in range(4):
                        ti = t0 + q * 4 + j
                        wwin = wid_of_tile[ti]
                        nc.tensor.matmul(
                            ps[:, j, :], ohs[:, q * 4 + j, :],
                            usb_w[wwin][:],
                            start=True, stop=True)
                    nc.scalar.copy(out=ub[:, q * 4:(q + 1) * 4, :], in_=ps[:])

                t = ep_pool.tile([128, kc, H], bf16, tag="t")
                nc.vector.tensor_tensor(out=t[:], in0=ub[:], in1=vg[:],
                                        op=mybir.AluOpType.add)
                nc.vector.tensor_scalar_max(out=t[:], in0=t[:], scalar1=0.0)
                m = ep_pool.tile([128, kc, H], fp16, tag="m")
                from concourse.bass import AP as _AP
                w2bc = _AP(w2b[:].tensor, w2b[:].offset, [[H, 128], [0, kc], [1, H]])
                nc.vector.tensor_tensor(out=m[:], in0=t[:], in1=w2bc,
                                        op=mybir.AluOpType.mult)
                w = H
                while w > 2:
                    half = w // 2
                    nc.vector.tensor_tensor(
                        out=m[:, :, 0:half], in0=m[:, :, 0:half],
                        in1=m[:, :, half:w], op=mybir.AluOpType.add)
                    w = half
                nc.vector.tensor_tensor(
                    out=racc[:, c * kc:(c + 1) * kc],
                    in0=m[:, :, 0], in1=m[:, :, 1], op=mybir.AluOpType.add)

            nc.vector.tensor_scalar_add(out=racc[:], in0=racc[:], scalar1=b2t[:, 0:1])
            nc.sync.dma_start(out=out[:], in_=racc[:])

    nc.compile()
    return nc


def _host_pack(row_l, col_l, rng_users):
    """Bin-pack local users into W_WIN windows (LPT greedy).
    Returns (slot_of_user [rng_users] -> table slot, max window load)."""
    counts = np.bincount(row_l, minlength=rng_users)
    order = np.argsort(-counts, kind="stable")
    loads = np.zeros(W_WIN, np.int64)
    fill = np.zeros(W_WIN, np.int64)   # user slots used per window
    win_of_user = np.empty(rng_users, np.int64)
    slot_of_user = np.empty(rng_users, np.int64)
    import heapq
    heap = [(0, w) for w in range(W_WIN)]
    heapq.heapify(heap)
    for u in order:
        cu = counts[u]
        # least-loaded window with a free user slot (lazy-deletion heap)
        while True:
            load, w = heapq.heappop(heap)
            if load == loads[w] and fill[w] < 128:
                break
        win_of_user[u] = w
        slot_of_user[u] = w * 128 + fill[w]
        fill[w] += 1
        loads[w] += cu
        if fill[w] < 128:
            heapq.heappush(heap, (loads[w], w))
    return slot_of_user, int(loads.max())


def _kernel_sel(z_user, z_item, row, col, W1, b1, W2, b2, pos):
    from concourse.bass_utils import run_bass_kernel_spmd
    global LAST_EXEC_TIME_NS, LAST_RESULTS
    import ml_dtypes
    E = row.shape[0]
    n_c = [len(p) for p in pos]

    # pack every core first so the shared schedule can adapt its capacity
    packs = []
    tpw = TPW
    for c in range(N_CORES):
        a, b = divmod(c, I_SPLIT)
        row_l = row[pos[c]] - a * U_RANGE
        slot_of_user, maxload = _host_pack(row_l, None, U_RANGE)
        packs.append(slot_of_user)
        tpw = max(tpw, -(-maxload // 128))

    # fixed schedule: windows 0..W_WIN-1, tpw tiles each; then trailing tiles
    base_tiles = W_WIN * tpw
    e_base = base_tiles * 128
    e_pad = -(-e_base // CHUNK) * CHUNK
    n_tiles_e = e_pad // 128
    wid_of_tile = [min(t // tpw, W_WIN - 1) for t in range(n_tiles_e)]

    in_maps = []
    recover = []
    for c in range(N_CORES):
        a, b = divmod(c, I_SPLIT)
        row_l = row[pos[c]] - a * U_RANGE
        col_l = col[pos[c]] - b * I_RANGE
        slot_of_user = packs[c]
        slots = slot_of_user[row_l]           # per-edge table slot
        winf = slots // 128                    # per-edge window
        lu = slots % 128
        # place edges: window w owns tile range [w*TPW, (w+1)*TPW)
        order = np.argsort(winf, kind="stable")
        # position within window
        ptr = np.zeros(W_WIN + 1, np.int64)
        wcnt = np.bincount(winf, minlength=W_WIN)
        ptr[1:] = np.cumsum(wcnt)
        # padded position: window w starts at w*TPW*128
        pos_in_win = np.empty(len(order), np.int64)
        pos_in_win[order] = np.arange(len(order)) - ptr[winf[order]]
        pad_pos = winf * (tpw * 128) + pos_in_win   # destination padded index
        # build arrays
        ohm = np.zeros((n_tiles_e, 128, 128), ml_dtypes.bfloat16)
        vloc = np.zeros(e_pad, np.int64)
        tile_i = pad_pos // 128
        col_i = pad_pos % 128
        ohm[tile_i, lu, col_i] = 1.0
        vloc[pad_pos] = col_l
        # permuted/padded user table
        zup = np.zeros((T_PAD2, HIDDEN), np.float32)
        zs = z_user[a * U_RANGE:(a + 1) * U_RANGE]
        zup[slot_of_user] = zs
        # wrap vidx
        wv = np.empty((128, e_pad // 16), np.int16)
        blk = vloc.astype(np.int16).reshape(e_pad // 16, 16).T
        for bb in range(8):
            wv[bb * 16:(bb + 1) * 16, :] = blk
        zi_p = np.concatenate(
            [z_item[b * I_RANGE:(b + 1) * I_RANGE],
             np.zeros((T_PAD - I_RANGE, HIDDEN), np.float32)])
        in_maps.append({
            "zu": zup, "zi": zi_p, "vidx": wv, "oh": ohm,
            "w1": W1, "b1r": b1.reshape(1, HIDDEN),
            "w2r": np.repeat(W2.reshape(1, HIDDEN), 128, axis=0),
            "b2r": np.full((128, 1), b2[0], np.float32),
        })
        recover.append(pad_pos)

    trace = os.environ.get("TRN_KERNEL_TRACE", "0") == "1"
    if trace:
        _maybe_install_ntff_hook()
    nc = _build_sel(e_pad, wid_of_tile)
    res = run_bass_kernel_spmd(nc, in_maps, core_ids=list(range(N_CORES)),
                               trace=trace)
    LAST_EXEC_TIME_NS = res.exec_time_ns
    LAST_RESULTS = res

    out_full = np.empty(E, np.float32)
    for c in range(N_CORES):
        oc = res.results[c]["out"]
        flat = oc.T.reshape(-1)
        out_full[pos[c]] = flat[recover[c]]
    return out_full.reshape(E, 1)


def kernel(z_user, z_item, row_idx, col_idx, W1, b1, W2, b2):
    global LAST_EXEC_TIME_NS, LAST_RESULTS
    from concourse.bass_utils import run_bass_kernel_spmd

    z_user = np.ascontiguousarray(np.asarray(z_user, dtype=np.float32))
    z_item = np.ascontiguousarray(np.asarray(z_item, dtype=np.float32))
    row = np.asarray(row_idx).astype(np.int64)
    col = np.asarray(col_idx).astype(np.int64)
    W1 = np.asarray(W1, dtype=np.float32)
    b1 = np.asarray(b1, dtype=np.float32)
    W2 = np.asarray(W2, dtype=np.float32)
    b2 = np.asarray(b2, dtype=np.float32)
    E = row.shape[0]

    # ---- host-side shard: assign each edge to core (row_range, col_range) ----
    core_of = (row // U_RANGE) * I_SPLIT + (col // I_RANGE)
    pos = [np.nonzero(core_of == c)[0] for c in range(N_CORES)]
    n_c = [len(p) for p in pos]
    e_pad = -(-max(n_c) // CHUNK) * CHUNK

    def wrap_idx(local_idx):
        full = np.zeros(e_pad, np.int16)
        full[:len(local_idx)] = local_idx.astype(np.int16)
        w = np.empty((128, e_pad // 16), np.int16)
        blk = full.reshape(e_pad // 16, 16).T  # [16, e_pad//16]
        for b in range(8):
            w[b * 16:(b + 1) * 16, :] = blk
        return w

    def pad_tbl(z):
        return np.concatenate(
            [z, np.zeros((T_PAD - z.shape[0], HIDDEN), np.float32)])

    b1row = b1.reshape(1, HIDDEN)
    w2rep = np.repeat(W2.reshape(1, HIDDEN), 128, axis=0)
    b2r = np.full((128, 1), b2[0], np.float32)

    in_maps = []
    for c in range(N_CORES):
        a, b = divmod(c, I_SPLIT)
        in_maps.append({
            "zu": pad_tbl(z_user[a * U_RANGE:(a + 1) * U_RANGE]),
            "zi": pad_tbl(z_item[b * I_RANGE:(b + 1) * I_RANGE]),
            "uidx": wrap_idx(row[pos[c]] - a * U_RANGE),
            "vidx": wrap_idx(col[pos[c]] - b * I_RANGE),
            "w1": W1, "b1r": b1row, "w2r": w2rep, "b2r": b2r,
        })

    if os.environ.get("TRN_KERNEL_VARIANT", "sel") == "sel":
        return _kernel_sel(z_user, z_item, row, col, W1, b1, W2, b2, pos)

    trace = os.environ.get("TRN_KERNEL_TRACE", "0") == "1"
    if trace:
        _maybe_install_ntff_hook()

    nc = _build(e_pad)
    res = run_bass_kernel_spmd(nc, in_maps, core_ids=list(range(N_CORES)),
                               trace=trace)
    LAST_EXEC_TIME_NS = res.exec_time_ns
    LAST_RESULTS = res

    out_full = np.empty(E, np.float32)
    for c in range(N_CORES):
        oc = res.results[c]["out"]  # [128, e_pad//128]; edge i at [i%128, i//128]
        flat = oc.T.reshape(-1)     # flat[i] = oc[i%128, i//128]
        out_full[pos[c]] = flat[:n_c[c]]
    return out_full.reshape(E, 1)
